# revision 1
# baseline (speedup 1.0000x reference)
"""Cond-LSTM Trainium2 kernel (nn_Cond_LSTM): batch-sharded SPMD over 8 NeuronCores.

Reference computation:
    f   = wei_F @ tnsr_cond                              (F,)
    WH  = einsum('ghf,f,gfk->ghk', wei_U, f, wei_V)      (4,H,H)
    xp  = einsum('ghi,tbi->tghb', wei_WI, tnsr_input)    (T,4,H,B)
    LSTM scan over T with z_t = xp_t + WH @ h_{t-1}
    out = stack([hs, cs])                                (2,T+1,H,B)

Sharding: data-parallel over batch (B=64 -> 8 per core); weights replicated.
All arithmetic runs on-device (bf16 matmuls, fp32 accumulation/activations).

Structure of the scan (per core):
  - z for each gate lives in its own PSUM bank (4 gates x 2 rotating banks
    = all 8 banks).  xp for a block of TBLK steps is accumulated into those
    banks ahead of time; the recurrent WH @ h matmuls then accumulate on
    top, and the activations read each gate's slice as soon as that gate's
    matmuls for the step have drained (per-gate tiles keep the dependency
    tracking fine-grained).
  - Gate matmul order per step is [f, i, g, o]; activations are issued
    per-gate (sigmoid f, sigmoid i, tanh g, sigmoid o) so the ACT engine
    overlaps the tail of the matmul burst.
  - The next block's xp GEMM, PSUM seeding and x-transpose DMAs are
    interleaved into the step loop so the PE fills the gaps while the
    activation/vector chain of the current step runs.
  - h/c outputs are staged in SBUF and written back with two DMAs per
    block instead of two per step.
"""

import numpy as np

T, B, I, H, F, C = 256, 64, 1024, 1024, 512, 512
NG = 4            # gates
N_CORES = 8
BL = B // N_CORES  # local batch = 8
TBLK = 8           # recurrence steps per x-transpose/xp block
NBLK = T // TBLK
KSUB = H // 128    # 8 k-tiles over hidden dim
ISUB = I // 128    # 8 k-tiles over input dim
MT = NG * H // 128  # 32 m-tiles over z rows
# gate order used on-device: [i, f, o, g]; reference order is [i, f, g, o]
GATE_PERM = [0, 1, 3, 2]
GI, GF, GO, GG = 0, 1, 2, 3          # device gate slots
MM_ORDER = [GF, GI, GG, GO]          # per-step matmul issue order
SEED_ORDER = [GI, GI, GF, GF, GO, GO, GG, GG]  # gate of xp m-chunk per trel

_CACHED = None


def _build_program(timing=False):
    import concourse.bacc as bacc
    import concourse.mybir as mybir
    import concourse.tile as tile

    F32 = mybir.dt.float32
    BF = mybir.dt.bfloat16
    F16 = mybir.dt.float16
    AF = mybir.ActivationFunctionType

    nc = bacc.Bacc("TRN2", target_bir_lowering=False, debug=False,
                   num_devices=N_CORES)

    # ---- per-core inputs (host-rearranged layouts; weights/x pre-cast
    # to bf16 on the host to halve the load DMA volume) ----
    x_in = nc.dram_tensor("x", [T * BL, I], BF, kind="ExternalInput")
    h0_in = nc.dram_tensor("h0c", [128, KSUB * BL], F32, kind="ExternalInput")
    c0_in = nc.dram_tensor("c0c", [128, KSUB * BL], F32, kind="ExternalInput")
    wf_in = nc.dram_tensor("wFp", [128, F // 128, C], F32, kind="ExternalInput")
    cond_in = nc.dram_tensor("condb", [128, C], F32, kind="ExternalInput")
    v_in = nc.dram_tensor("Vp", [128, F // 128, NG, H], BF, kind="ExternalInput")
    ut_in = nc.dram_tensor("UTp", [128, F // 128, NG, H], BF, kind="ExternalInput")
    wit_in = nc.dram_tensor("WITp", [128, ISUB, NG * H], BF,
                            kind="ExternalInput")
    if timing:
        out = nc.dram_tensor("out", [2, T + 1, H, BL], F32)
        tok = nc.dram_tensor("tok", [128, KSUB * BL], F32, kind="ExternalOutput")
    else:
        out = nc.dram_tensor("out", [2, T + 1, H, BL], F32, kind="ExternalOutput")
        tok = None

    FS = F // 128  # 4

    with tile.TileContext(nc) as tc:
        # ---------- resident weights ----------
        with tc.tile_pool(name="resident", bufs=1) as res:
            wht = res.tile([128, KSUB, NG * H], BF)   # lhsT for WH part (8 MB)
            wit = res.tile([128, ISUB, NG * H], BF)   # lhsT for xp part (8 MB)

            # ---------- phase 0a: f = rowsum(wFp * cond) ----------
            # pipelined per f-chunk (DMA / multiply / reduce) so fvec --
            # which gates the whole WHT build -- is ready sooner
            with tc.tile_pool(name="p0a", bufs=1) as p0:
                cond_sb = p0.tile([128, C], F32)
                nc.sync.dma_start(cond_sb[:], cond_in[:, :])
                wf_sb = p0.tile([128, FS, C], F32)
                prod = p0.tile([128, FS, C], F32)
                fvec = p0.tile([128, FS], F32)

                # ---------- phase 0b: WHT[g] = Vf[g]^T-contracted with UT[g] ----
                with tc.tile_pool(name="p0b", bufs=2) as pb, \
                     tc.tile_pool(name="p0ps", bufs=4, space="PSUM") as pps:
                    # fvec chunks interleaved with gate 0's loads/multiplies
                    # (DVE runs its queue in order; emitting all fvec ops
                    # first would delay gate 0's first vf chunk and with it
                    # the first build matmul)
                    vg0 = pb.tile([128, FS, H], BF, tag="vg")
                    utg0 = pb.tile([128, FS, H], BF, tag="utg", bufs=3)
                    vf0 = pb.tile([128, FS, H], BF, tag="vf")
                    for fs in range(FS):
                        nc.sync.dma_start(wf_sb[:, fs], wf_in[:, fs, :])
                        nc.sync.dma_start(vg0[:, fs], v_in[:, fs, 0, :])
                        nc.sync.dma_start(utg0[:, fs], ut_in[:, fs, 0, :])
                        nc.vector.tensor_tensor(
                            prod[:, fs], wf_sb[:, fs], cond_sb[:],
                            mybir.AluOpType.mult)
                        nc.vector.reduce_sum(
                            fvec[:, fs:fs + 1], prod[:, fs],
                            axis=mybir.AxisListType.X)
                        nc.vector.tensor_tensor(
                            vf0[:, fs], vg0[:, fs],
                            fvec[:, fs, None].to_broadcast((128, H)),
                            mybir.AluOpType.mult)
                    for g in range(NG):
                        if g == 0:
                            vg, utg, vf = vg0, utg0, vf0
                        else:
                            vg = pb.tile([128, FS, H], BF, tag="vg")
                            utg = pb.tile([128, FS, H], BF, tag="utg", bufs=3)
                            vf = pb.tile([128, FS, H], BF, tag="vf")
                            # per-fs loads + multiplies: the build matmuls
                            # accumulate over fs, so the first matmul can
                            # start once the fs=0 slices land
                            for fs in range(FS):
                                nc.sync.dma_start(vg[:, fs], v_in[:, fs, g, :])
                                nc.sync.dma_start(utg[:, fs],
                                                  ut_in[:, fs, g, :])
                                nc.vector.tensor_tensor(
                                    vf[:, fs], vg[:, fs],
                                    fvec[:, fs, None].to_broadcast((128, H)),
                                    mybir.AluOpType.mult)
                        for kh in range(KSUB):
                            for mch in range(2):
                                ps = pps.tile([128, 512], F32, tag="whps")
                                for fs in range(FS):
                                    nc.tensor.matmul(
                                        ps[:],
                                        vf[:, fs, kh * 128:(kh + 1) * 128],
                                        utg[:, fs, mch * 512:(mch + 1) * 512],
                                        start=(fs == 0), stop=(fs == FS - 1))
                                nc.scalar.copy(
                                    wht[:, kh,
                                        g * H + mch * 512:g * H + (mch + 1) * 512],
                                    ps[:])

            # ---------- phase 0c: load WI^T (bf16); split into per-chunk
            # DMAs so they interleave with the U/V loads on the DMA
            # engines instead of hogging them in one long transfer ----
            for ic in range(ISUB):
                nc.sync.dma_start(wit[:, ic, :], wit_in[:, ic, :])

            # ---------- copy h0/c0 rows of the output ----------
            with tc.tile_pool(name="p0e", bufs=1) as pe:
                h0_sb = pe.tile([128, KSUB, BL], F32)
                nc.sync.dma_start(h0_sb[:], h0_in[:, :].rearrange(
                    "p (k b) -> p k b", k=KSUB))
                c0_sb = pe.tile([128, KSUB, BL], F32)
                nc.sync.dma_start(c0_sb[:], c0_in[:, :].rearrange(
                    "p (k b) -> p k b", k=KSUB))
                nc.sync.dma_start(
                    out[0, 0].rearrange("(k p) b -> p k b", p=128), h0_sb[:])
                nc.sync.dma_start(
                    out[1, 0].rearrange("(k p) b -> p k b", p=128), c0_sb[:])

                # ---------- phase 1: the scan ----------
                with tc.tile_pool(name="xt", bufs=2 * ISUB) as xt_pool, \
                     tc.tile_pool(name="act", bufs=4) as act, \
                     tc.tile_pool(name="hc", bufs=3) as hc, \
                     tc.tile_pool(name="zps", bufs=2, space="PSUM") as zpool:

                    def load_xtb(blk, ic):
                        # bf16 transpose of one 128-col slice of x
                        xt = xt_pool.tile([128, TBLK * BL], BF,
                                          name=f"xb{ic}", tag=f"xb{ic}",
                                          bufs=2)
                        nc.sync.dma_start_transpose(
                            xt[:],
                            x_in[blk * TBLK * BL:(blk + 1) * TBLK * BL,
                                 ic * 128:(ic + 1) * 128])
                        return xt

                    def gate_chunk(xtb, ic, dep):
                        # identity DVE copy with a dummy `bypass` dependency
                        # on the current step's h: the xp chunk only becomes
                        # ready right as the consuming step starts, so the
                        # scheduler cannot front-load the next-block GEMM
                        # into earlier steps' idle PE windows (the transpose
                        # DMAs themselves complete far ahead of the steps)
                        xc = xt_pool.tile([128, TBLK * BL], BF,
                                          name=f"xc{ic}", tag=f"xc{ic}",
                                          bufs=2)
                        nc.vector.tensor_tensor(
                            xc[:], xtb[:], dep, mybir.AluOpType.bypass)
                        return xc

                    def z_view(ztiles, gi):
                        # gates f,i share one 2-bank tile (ztiles[0]) so a
                        # single ACT sigmoid covers both; g,o have their own
                        if gi == GF:
                            return ztiles[0][:, 0]
                        if gi == GI:
                            return ztiles[0][:, 1]
                        return ztiles[1] if gi == GG else ztiles[2]

                    def xp_ic(ztiles, xt, ic):
                        # one x-column chunk into all 32 z m-tiles; the first
                        # write of each gate's bank carries start=True (clears
                        # the whole bank's has_written)
                        for gi in MM_ORDER:
                            zv = z_view(ztiles, gi)
                            for mrel in range(KSUB):
                                m = gi * KSUB + mrel
                                nc.tensor.matmul(
                                    zv[:, mrel].rearrange("p t b -> p (t b)"),
                                    wit[:, ic, m * 128:(m + 1) * 128],
                                    xt[:],
                                    start=(ic == 0 and mrel == 0), stop=False)

                    def alloc_z():
                        zfi = zpool.tile([128, 2, KSUB, TBLK, BL], F32,
                                         name="zfi", tag="zfi")
                        zg = zpool.tile([128, KSUB, TBLK, BL], F32,
                                        name="zg", tag="zg")
                        zo = zpool.tile([128, KSUB, TBLK, BL], F32,
                                        name="zo", tag="zo")
                        return [zfi, zg, zo]

                    # prologue: block 0 fully prepped (no copy-gating needed
                    # there); block 1's transposes loaded, first chunk copied
                    xtb0 = [load_xtb(0, ic) for ic in range(ISUB)]
                    z_cur = alloc_z()
                    for ic in range(ISUB):
                        xp_ic(z_cur, xtb0[ic], ic)
                    xtb_cur = [load_xtb(1, ic) for ic in range(ISUB)]
                    xtb_nxt = [None] * ISUB
                    xtc_cur = [None] * ISUB
                    xtc_nxt = [None] * ISUB

                    hprev = hc.tile([128, KSUB, BL], BF, tag="hb")
                    nc.vector.tensor_copy(hprev[:], h0_sb[:])
                    xtc_cur[0] = gate_chunk(
                        xtb_cur[0], 0, hprev[:].rearrange("p k b -> p (k b)"))
                    c_prev = c0_sb[:]               # [128, KSUB, BL] f32 view

                    for blk in range(NBLK):
                        z_next = alloc_z() if blk + 1 < NBLK else None
                        for trel in range(TBLK):
                            # ---- recurrent matmuls, gate-major [f,i,g,o] ----
                            for gi in MM_ORDER:
                                zv = z_view(z_cur, gi)
                                for mrel in range(KSUB):
                                    m = gi * KSUB + mrel
                                    for kt in range(KSUB):
                                        stop = (trel == TBLK - 1
                                                and mrel == KSUB - 1
                                                and kt == KSUB - 1)
                                        nc.tensor.matmul(
                                            zv[:, mrel, trel, :],
                                            wht[:, kt, m * 128:(m + 1) * 128],
                                            hprev[:, kt, :],
                                            start=False, stop=stop)
                            # ---- interleaved next-block xp: chunk t was
                            # copy-gated on DVE during step t-1
                            if blk + 1 < NBLK:
                                xp_ic(z_next, xtc_cur[trel], trel)
                            # ---- activation / vector chain (gate values in
                            # fp16 so the DVE 2-byte fast modes apply) ----
                            sfi = act.tile([128, 2 * KSUB * BL], F16,
                                           tag="sfi")
                            nc.scalar.activation(
                                sfi[:].rearrange("p (g m b) -> p g m b",
                                                 g=2, m=KSUB),
                                z_cur[0][:, :, :, trel, :], AF.Sigmoid)
                            tg = act.tile([128, KSUB * BL], F16, tag="tg")
                            nc.scalar.activation(
                                tg[:].rearrange("p (m b) -> p m b", m=KSUB),
                                z_cur[1][:, :, trel, :], AF.Tanh)
                            so = act.tile([128, KSUB * BL], F16, tag="so")
                            nc.scalar.activation(
                                so[:].rearrange("p (m b) -> p m b", m=KSUB),
                                z_cur[2][:, :, trel, :], AF.Sigmoid)
                            cnew = hc.tile([128, KSUB * BL], F16, tag="c")
                            nc.vector.tensor_mul(
                                cnew[:], sfi[:, :KSUB * BL],
                                c_prev.rearrange("p k b -> p (k b)"))
                            ig = act.tile([128, KSUB * BL], F16, tag="ig")
                            nc.vector.tensor_mul(
                                ig[:], sfi[:, KSUB * BL:], tg[:])
                            nc.vector.tensor_add(cnew[:], cnew[:], ig[:])
                            tcn = act.tile([128, KSUB * BL], F16, tag="tc")
                            nc.scalar.activation(tcn[:], cnew[:], AF.Tanh)
                            cf32 = act.tile([128, KSUB * BL], F32, tag="c32")
                            nc.vector.tensor_copy(cf32[:], cnew[:])
                            # bf16 h for the next step first (critical path),
                            # f32 copy for the output afterwards
                            hb = hc.tile([128, KSUB, BL], BF, tag="hb")
                            nc.vector.tensor_mul(
                                hb[:].rearrange("p k b -> p (k b)"),
                                so[:], tcn[:])
                            hnew = act.tile([128, KSUB * BL], F32, tag="hnew")
                            nc.vector.tensor_mul(hnew[:], so[:], tcn[:])
                            # copy-gate the next chunk (made dependent on
                            # this step's h so it cannot become ready early)
                            hbdep = hb[:].rearrange("p k b -> p (k b)")
                            j = trel + 1
                            if blk + 1 < NBLK and j < ISUB:
                                xtc_cur[j] = gate_chunk(xtb_cur[j], j, hbdep)
                            elif blk + 2 < NBLK and j >= ISUB:
                                xtc_nxt[0] = gate_chunk(xtb_nxt[0], 0, hbdep)
                            t = blk * TBLK + trel
                            # per-step output DMAs; the wait on hnew paces the
                            # SP queue, so the x-transposes for block blk+2
                            # issued right after stay spread out
                            nc.sync.dma_start(
                                out[0, t + 1].rearrange("(k p) b -> p k b",
                                                        p=128),
                                hnew[:].rearrange("p (k b) -> p k b", k=KSUB))
                            nc.sync.dma_start(
                                out[1, t + 1].rearrange("(k p) b -> p k b",
                                                        p=128),
                                cf32[:].rearrange("p (k b) -> p k b", k=KSUB))
                            if blk + 2 < NBLK:
                                xtb_nxt[trel] = load_xtb(blk + 2, trel)
                            hprev = hb
                            c_prev = cnew[:].rearrange("p (k b) -> p k b",
                                                       k=KSUB)
                        z_cur = z_next
                        xtb_cur = xtb_nxt
                        xtb_nxt = [None] * ISUB
                        xtc_cur = xtc_nxt
                        xtc_nxt = [None] * ISUB
                    if timing:
                        nc.sync.dma_start(tok[:, :], hnew[:])
    nc.compile()
    return nc


def _prep_in_maps(tnsr_input, h0, c0, tnsr_cond, wei_F, wei_U, wei_V, wei_WI):
    import ml_dtypes
    perm = GATE_PERM
    f32 = np.float32
    bf16 = ml_dtypes.bfloat16
    # weights: replicated, host-side layout rearranges + dtype casts only
    wFp = np.ascontiguousarray(
        wei_F.reshape(F // 128, 128, C).transpose(1, 0, 2)).astype(f32)
    condb = np.ascontiguousarray(
        np.broadcast_to(tnsr_cond[None, :], (128, C))).astype(f32)
    # Vp[p, fs, g, h] = V[g, fs*128+p, h]
    Vp = np.ascontiguousarray(
        wei_V[perm].reshape(NG, F // 128, 128, H).transpose(2, 1, 0, 3)).astype(bf16)
    # UTp[p, fs, g, h] = U[g, h, fs*128+p]
    UTp = np.ascontiguousarray(
        wei_U[perm].reshape(NG, H, F // 128, 128).transpose(3, 2, 0, 1)).astype(bf16)
    # WITp[p, isub, g*H + zh] = WI[g, zh, isub*128+p]
    WITp = np.ascontiguousarray(
        wei_WI[perm].reshape(NG, H, ISUB, 128).transpose(3, 2, 0, 1)
        .reshape(128, ISUB, NG * H)).astype(bf16)

    in_maps = []
    for cix in range(N_CORES):
        bsl = slice(cix * BL, (cix + 1) * BL)
        xs = np.ascontiguousarray(
            tnsr_input[:, bsl, :].reshape(T * BL, I)).astype(bf16)
        h0c = np.ascontiguousarray(
            h0[:, bsl].reshape(KSUB, 128, BL).transpose(1, 0, 2)
            .reshape(128, KSUB * BL)).astype(f32)
        c0c = np.ascontiguousarray(
            c0[:, bsl].reshape(KSUB, 128, BL).transpose(1, 0, 2)
            .reshape(128, KSUB * BL)).astype(f32)
        in_maps.append({
            "x": xs, "h0c": h0c, "c0c": c0c, "wFp": wFp, "condb": condb,
            "Vp": Vp, "UTp": UTp, "WITp": WITp,
        })
    return in_maps


def kernel(**inputs):
    global _CACHED
    from concourse.bass_utils import run_bass_kernel_spmd

    if _CACHED is None:
        _CACHED = _build_program()
    nc = _CACHED

    in_maps = _prep_in_maps(
        np.asarray(inputs["tnsr_input"]), np.asarray(inputs["h0"]),
        np.asarray(inputs["c0"]), np.asarray(inputs["tnsr_cond"]),
        np.asarray(inputs["wei_F"]), np.asarray(inputs["wei_U"]),
        np.asarray(inputs["wei_V"]), np.asarray(inputs["wei_WI"]))

    res = run_bass_kernel_spmd(nc, in_maps, list(range(N_CORES)))
    full = np.empty((2, T + 1, H, B), dtype=np.float32)
    for cix in range(N_CORES):
        full[:, :, :, cix * BL:(cix + 1) * BL] = res.results[cix]["out"]
    return full

